# revision 5
# baseline (speedup 1.0000x reference)
"""Trainium2 Bass kernel for nn_NodeEncoder (per-type Linear over interleaved node types).

Problem: x [800000, 128] f32, W [8, 256, 128], b [8, 256].
Node n has type k = n % 8; y[n] = (W[k] * mask_k) @ x[n] + b[k], y [800000, 256].

Strategy (8 cores, data-parallel over graphs, weights replicated):
  - Each core gets 100000 consecutive nodes (12500 per type), padded to
    12544 = 28*448 nodes per type.
  - Weight-stationary matmuls: lhsT = W[k] half [kk, 128 feats] (tiny),
    rhs = x streamed [kk, 512 nodes] per chunk, out = PSUM [128 feats,
    512 nodes] fp32 (one full 2 KiB bank).  25 chunks (24x512 + 256)
    per (type, feat-half); 400 matmuls per core total.
  - x ships fp16, type-major: xd[R_OFF[k] + d, i] = x_typek[i, d], each
    type's block fully contiguous -> 8 large input DMAs per core with
    maximal descriptors (25 KiB per partition row).
  - For types with dim < 128 a ones-row is appended (bias rides as an
    extra contraction row of the weight tile).  Types 3,7 (dim 128) get
    their bias during PSUM eviction: ACT activation-bias / DVE
    tensor_scalar_add with a per-partition [128,1] bias vector.
  - Evictions (PSUM fp32 -> SBUF fp16) alternate Scalar:Vector 5:4
    (matching their 1.2 / 0.96 GHz rates).  Output y is written
    feat-major [16 blocks (k,h), 128 feats, 12544 nodes] fp16, each
    (k,h) block one contiguous 3.2 MiB DMA issued from the (otherwise
    idle) GpSimd queue; the host untangles to node-major fp32.
"""

import os
import sys

import numpy as np

for _p in ("/root/.axon_site", "/root/.axon_site/_ro/trn_rl_repo", "/root/.axon_site/_ro/pypackages"):
    if os.path.isdir(_p) and _p not in sys.path:
        sys.path.append(_p)

import concourse.bass as bass
import concourse.mybir as mybir
import concourse.tile as tile
from concourse import bacc
from concourse.bass_utils import run_bass_kernel_spmd

N_TYPES = 8
MAX_DIM = 128
FEAT = 256
N_GRAPHS = 100000
NODE_DIMS = np.array([16, 32, 64, 128, 64, 32, 16, 128], dtype=np.int32)

N_CORES = 8
NODES_PER_CORE = N_GRAPHS * N_TYPES // N_CORES  # 100000
NPT_REAL = NODES_PER_CORE // N_TYPES            # 12500 nodes per type per core
NPT = 12544                                     # padded: 28 * 448 = 24.5 * 512
CHUNKS = [512] * 24 + [256]                     # sum = 12544

_F32 = mybir.dt.float32
_F16 = mybir.dt.float16

# kk = contraction rows per type: dim + 1 (ones-row folds the bias) for
# dim < 128; types 3,7 use all 128 rows and get bias at eviction.
FOLD = [int(d) < MAX_DIM for d in NODE_DIMS]
KK = [int(d) + (1 if f else 0) for d, f in zip(NODE_DIMS, FOLD)]
R_OFF = np.concatenate([[0], np.cumsum(KK)]).astype(int)
R_TOT = int(R_OFF[-1])                          # 486
BIDX = {(3, 0): 0, (3, 1): 1, (7, 0): 2, (7, 1): 3}
TYPE_ORDER = [0, 6, 1, 5, 2, 4, 3, 7]           # small input blocks first

_nc_cache = {}


def _build_nc():
    if "nc" in _nc_cache:
        return _nc_cache["nc"]
    nc = bacc.Bacc("TRN2", target_bir_lowering=False, debug=False)
    xd = nc.dram_tensor("xd", [R_TOT, NPT], _F16, kind="ExternalInput").ap()
    wtb = nc.dram_tensor("wtb", [128, 2 * N_TYPES * 128], _F16, kind="ExternalInput").ap()
    bvec = nc.dram_tensor("bvec", [128, 4], _F32, kind="ExternalInput").ap()
    y = nc.dram_tensor("y", [2 * N_TYPES, 128, NPT], _F16, kind="ExternalOutput").ap()

    ident = mybir.ActivationFunctionType.Identity

    # Per (type, half): 8 groups of 3x512-col matmul chunks (one 3-bank PSUM
    # tile each, single batched eviction) + a 256-col tail.  Output DMA per
    # group; the tail rides with the last group's DMA.
    GROUPS = [(g * 1536, 1536) for g in range(8)] + [(12288, 256)]
    XSPLIT = 6144  # input DMA split point (group boundary)

    with tile.TileContext(nc) as tc:
        with (
            tc.tile_pool(name="const", bufs=1) as const,
            tc.tile_pool(name="xin", bufs=2) as xin_pool,
            tc.tile_pool(name="outsb", bufs=2) as out_pool,
            tc.tile_pool(name="ps", bufs=2, space="PSUM") as ps_pool,
            tc.tile_pool(name="pst", bufs=2, space="PSUM") as pst_pool,
        ):
            wtb_sb = const.tile([128, 2 * N_TYPES * 128], _F16)
            nc.sync.dma_start(wtb_sb[:], wtb[:])
            bv_sb = const.tile([128, 4], _F32)
            nc.sync.dma_start(bv_sb[:], bvec[:])

            ev = 0  # global eviction counter: 6:5 Scalar:Vector split
            for k in TYPE_ORDER:
                kk = KK[k]
                xs = xin_pool.tile([128, NPT], _F16, tag="xs", name=f"xs_{k}")
                nc.sync.dma_start(xs[0:kk, 0:XSPLIT],
                                  xd[R_OFF[k]:R_OFF[k] + kk, 0:XSPLIT])
                nc.sync.dma_start(xs[0:kk, XSPLIT:NPT],
                                  xd[R_OFF[k]:R_OFF[k] + kk, XSPLIT:NPT])
                out_sb = out_pool.tile([128, 2 * NPT], _F16, tag="os", name=f"os_{k}")
                for h in range(2):
                    w_ap = wtb_sb[0:kk, (2 * k + h) * 128:(2 * k + h + 1) * 128]
                    for gi, (goff, gw) in enumerate(GROUPS):
                        tail = gw < 1536
                        pool = pst_pool if tail else ps_pool
                        ps = pool.tile([128, 256 if tail else 1536], _F32,
                                       tag="pst" if tail else "ps",
                                       name=f"ps_{k}_{h}_{gi}")
                        for off in range(0, gw, 512):
                            cw = min(512, gw - off)
                            nc.tensor.matmul(
                                ps[:, off:off + cw], w_ap,
                                xs[0:kk, goff + off:goff + off + cw],
                                start=True, stop=True,
                            )
                        dst = out_sb[:, h * NPT + goff:h * NPT + goff + gw]
                        use_act = (ev % 11) < 6
                        ev += 1
                        if k in (3, 7):
                            j = BIDX[(k, h)]
                            if use_act:
                                nc.scalar.activation(dst, ps[:, 0:gw], ident,
                                                     bias=bv_sb[:, j:j + 1])
                            else:
                                nc.vector.tensor_scalar_add(dst, ps[:, 0:gw],
                                                            bv_sb[:, j:j + 1])
                        else:
                            if use_act:
                                nc.scalar.copy(dst, ps[:, 0:gw])
                            else:
                                nc.vector.tensor_copy(dst, ps[:, 0:gw])
                        # output DMA: one per two groups (3072 cols ~ 786 KiB,
                        # keeps DMA records large); the tail rides with the
                        # last pair (issued after all three evicts).
                        if gi in (1, 3, 5):
                            doff = goff - 1536
                            nc.gpsimd.dma_start(
                                y[2 * k + h][:, doff:doff + 3072],
                                out_sb[:, h * NPT + doff:h * NPT + doff + 3072])
                        elif gi == 8:
                            nc.gpsimd.dma_start(
                                y[2 * k + h][:, 9216:NPT],
                                out_sb[:, h * NPT + 9216:h * NPT + NPT])

    nc.finalize()
    _nc_cache["nc"] = nc
    return nc


def _prep_weights(W, b):
    mask = (np.arange(MAX_DIM)[None, None, :] < NODE_DIMS[:, None, None])
    W_eff = np.where(mask, W, 0).astype(np.float32)  # [T, F, D]
    wtb = np.zeros((128, 2 * N_TYPES * 128), dtype=np.float32)
    for k in range(N_TYPES):
        d = int(NODE_DIMS[k])
        for h in range(2):
            c0 = (2 * k + h) * 128
            wtb[0:d, c0:c0 + 128] = W_eff[k, h * 128:(h + 1) * 128, :d].T
            if FOLD[k]:
                wtb[d, c0:c0 + 128] = b[k, h * 128:(h + 1) * 128]
    bvec = np.zeros((128, 4), dtype=np.float32)
    for (k, h), j in BIDX.items():
        bvec[:, j] = b[k, h * 128:(h + 1) * 128]
    return wtb.astype(np.float16), bvec


def _prep_x_shard(x, c):
    """fp16 type-major transposed layout: xd[R_OFF[k]+d, i] = x_k[i, d]
    where x_k[i] = x[c*100000 + 8*i + k] (node i of type k on core c),
    with a ones-row at d = dim_k for the bias-folding types."""
    xc = x[c * NODES_PER_CORE:(c + 1) * NODES_PER_CORE]
    xd = np.zeros((R_TOT, NPT), dtype=np.float16)
    for k in range(N_TYPES):
        d = int(NODE_DIMS[k])
        xk = xc[k::N_TYPES, :d]                       # [12500, d] f32
        xd[R_OFF[k]:R_OFF[k] + d, :NPT_REAL] = xk.astype(np.float16).T
        if FOLD[k]:
            xd[R_OFF[k] + d, :NPT_REAL] = 1.0
    return xd


def run(x, W, b, trace=False):
    nc = _build_nc()
    wtb, bvec = _prep_weights(W, b)
    in_maps = []
    for c in range(N_CORES):
        in_maps.append({
            "xd": _prep_x_shard(x, c),
            "wtb": wtb,
            "bvec": bvec,
        })
    res = run_bass_kernel_spmd(nc, in_maps, list(range(N_CORES)), trace=trace)
    y = np.empty((N_GRAPHS * N_TYPES, FEAT), dtype=np.float32)
    for c in range(N_CORES):
        yd = np.asarray(res.results[c]["y"]).astype(np.float32)
        yd = yd.reshape(N_TYPES, 2, 128, NPT)         # [k, h, p, i]
        yc = yd.transpose(3, 0, 1, 2).reshape(NPT, N_TYPES, FEAT)[:NPT_REAL]
        y[c * NODES_PER_CORE:(c + 1) * NODES_PER_CORE] = yc.reshape(
            NODES_PER_CORE, FEAT)
    return y, res


def kernel(**inputs):
    y, _ = run(inputs["x"], inputs["W"], inputs["b"])
    return y


if __name__ == "__main__":
    rng = np.random.default_rng(0)
    x = rng.standard_normal((N_GRAPHS * N_TYPES, MAX_DIM), dtype=np.float32)
    W = (rng.standard_normal((N_TYPES, FEAT, MAX_DIM), dtype=np.float32) * 0.05)
    b = (rng.standard_normal((N_TYPES, FEAT), dtype=np.float32) * 0.05)
    y, res = run(x, W, b)
    mask = (np.arange(MAX_DIM)[None, None, :] < NODE_DIMS[:, None, None])
    W_eff = np.where(mask, W, 0).astype(np.float32)
    idx = rng.integers(0, N_GRAPHS * N_TYPES, 256)
    exp = np.stack([W_eff[n % 8] @ x[n] + b[n % 8] for n in idx])
    act = y[idx]
    err = np.abs(act - exp).max() / (np.abs(exp).max() + 1e-30)
    print("spot-check rel err:", err)


# revision 6
# speedup vs baseline: 1.2103x; 1.2103x over previous
"""Trainium2 Bass kernel for nn_NodeEncoder (per-type Linear over interleaved node types).

Problem: x [800000, 128] f32, W [8, 256, 128], b [8, 256].
Node n has type k = n % 8; y[n] = (W[k] * mask_k) @ x[n] + b[k], y [800000, 256].

Strategy (8 cores, data-parallel over graphs, weights replicated):
  - Each core gets 100000 consecutive nodes (12500 per type), padded to
    12544 = 28*448 nodes per type.
  - Weight-stationary matmuls: lhsT = W[k] half [kk, 128 feats] (tiny),
    rhs = x streamed [kk, 512 nodes] per chunk, out = PSUM [128 feats,
    512 nodes] fp32 (one full 2 KiB bank).  25 chunks (24x512 + 256)
    per (type, feat-half); 400 matmuls per core total.
  - x ships fp16, type-major: xd[R_OFF[k] + d, i] = x_typek[i, d], each
    type's block fully contiguous -> 8 large input DMAs per core with
    maximal descriptors (25 KiB per partition row).
  - For types with dim < 128 a ones-row is appended (bias rides as an
    extra contraction row of the weight tile).  Types 3,7 (dim 128) get
    their bias during PSUM eviction: ACT activation-bias / DVE
    tensor_scalar_add with a per-partition [128,1] bias vector.
  - Evictions (PSUM fp32 -> SBUF fp16) alternate Scalar:Vector 5:4
    (matching their 1.2 / 0.96 GHz rates).  Output y is written
    feat-major [16 blocks (k,h), 128 feats, 12544 nodes] fp16, each
    (k,h) block one contiguous 3.2 MiB DMA issued from the (otherwise
    idle) GpSimd queue; the host untangles to node-major fp32.
"""

import os
import sys

import numpy as np

for _p in ("/root/.axon_site", "/root/.axon_site/_ro/trn_rl_repo", "/root/.axon_site/_ro/pypackages"):
    if os.path.isdir(_p) and _p not in sys.path:
        sys.path.append(_p)

import concourse.bass as bass
import concourse.mybir as mybir
import concourse.tile as tile
from concourse import bacc
from concourse.bass_utils import run_bass_kernel_spmd

N_TYPES = 8
MAX_DIM = 128
FEAT = 256
N_GRAPHS = 100000
NODE_DIMS = np.array([16, 32, 64, 128, 64, 32, 16, 128], dtype=np.int32)

N_CORES = 8
NODES_PER_CORE = N_GRAPHS * N_TYPES // N_CORES  # 100000
NPT_REAL = NODES_PER_CORE // N_TYPES            # 12500 nodes per type per core
NPT = 12544                                     # padded: 28 * 448 = 24.5 * 512
CHUNKS = [512] * 24 + [256]                     # sum = 12544

_F32 = mybir.dt.float32
_F16 = mybir.dt.float16

# kk = contraction rows per type: dim + 1 (ones-row folds the bias) for
# dim < 128; types 3,7 use all 128 rows and get bias at eviction.
FOLD = [int(d) < MAX_DIM for d in NODE_DIMS]
KK = [int(d) + (1 if f else 0) for d, f in zip(NODE_DIMS, FOLD)]
R_OFF = np.concatenate([[0], np.cumsum(KK)]).astype(int)
R_TOT = int(R_OFF[-1])                          # 486
BIDX = {(3, 0): 0, (3, 1): 1, (7, 0): 2, (7, 1): 3}
TYPE_ORDER = [0, 6, 1, 5, 2, 4, 3, 7]           # small input blocks first

_nc_cache = {}


def _build_nc():
    if "nc" in _nc_cache:
        return _nc_cache["nc"]
    nc = bacc.Bacc("TRN2", target_bir_lowering=False, debug=False)
    xd = nc.dram_tensor("xd", [R_TOT, NPT], _F16, kind="ExternalInput").ap()
    wtb = nc.dram_tensor("wtb", [128, 2 * N_TYPES * 128], _F16, kind="ExternalInput").ap()
    bvec = nc.dram_tensor("bvec", [128, 4], _F32, kind="ExternalInput").ap()
    y = nc.dram_tensor("y", [2 * N_TYPES, 128, NPT], _F16, kind="ExternalOutput").ap()

    ident = mybir.ActivationFunctionType.Identity

    # Per (type, half): 12 PSUM tiles of 2x512-col matmul chunks + a 256-col
    # tail (which borrows a quarter of a pool tile).  One batched eviction
    # per tile; one output DMA per 3 tiles (~786 KiB).
    GROUPS = [(g * 1024, 1024) for g in range(12)] + [(12288, 256)]
    XSPLIT = 6144  # input DMA split point (tile-group boundary)

    with tile.TileContext(nc) as tc:
        with (
            tc.tile_pool(name="const", bufs=1) as const,
            tc.tile_pool(name="xin", bufs=2) as xin_pool,
            tc.tile_pool(name="outsb", bufs=4) as out_pool,
            tc.tile_pool(name="ps", bufs=4, space="PSUM") as ps_pool,
        ):
            wtb_sb = const.tile([128, 2 * N_TYPES * 128], _F16)
            nc.sync.dma_start(wtb_sb[:], wtb[:])
            bv_sb = const.tile([128, 4], _F32)
            nc.sync.dma_start(bv_sb[:], bvec[:])

            ev = 0  # global eviction counter: 6:5 Scalar:Vector split
            for k in TYPE_ORDER:
                kk = KK[k]
                xs = xin_pool.tile([128, NPT], _F16, tag="xs", name=f"xs_{k}")
                nc.sync.dma_start(xs[0:kk, 0:XSPLIT],
                                  xd[R_OFF[k]:R_OFF[k] + kk, 0:XSPLIT])
                nc.sync.dma_start(xs[0:kk, XSPLIT:NPT],
                                  xd[R_OFF[k]:R_OFF[k] + kk, XSPLIT:NPT])
                for h in range(2):
                    out_sb = out_pool.tile([128, NPT], _F16, tag="os",
                                           name=f"os_{k}_{h}")
                    w_ap = wtb_sb[0:kk, (2 * k + h) * 128:(2 * k + h + 1) * 128]
                    for gi, (goff, gw) in enumerate(GROUPS):
                        ps = ps_pool.tile([128, 1024], _F32, tag="ps",
                                          name=f"ps_{k}_{h}_{gi}")
                        for off in range(0, gw, 512):
                            cw = min(512, gw - off)
                            nc.tensor.matmul(
                                ps[:, off:off + cw], w_ap,
                                xs[0:kk, goff + off:goff + off + cw],
                                start=True, stop=True,
                            )
                        dst = out_sb[:, goff:goff + gw]
                        use_act = (ev % 11) < 6
                        ev += 1
                        if k in (3, 7):
                            j = BIDX[(k, h)]
                            if use_act:
                                nc.scalar.activation(dst, ps[:, 0:gw], ident,
                                                     bias=bv_sb[:, j:j + 1])
                            else:
                                nc.vector.tensor_scalar_add(dst, ps[:, 0:gw],
                                                            bv_sb[:, j:j + 1])
                        else:
                            if use_act:
                                nc.scalar.copy(dst, ps[:, 0:gw])
                            else:
                                nc.vector.tensor_copy(dst, ps[:, 0:gw])
                        # output DMA per 3 tiles; tail rides with the last.
                        if gi in (2, 5, 8):
                            doff = goff - 2048
                            nc.gpsimd.dma_start(
                                y[2 * k + h][:, doff:doff + 3072],
                                out_sb[:, doff:doff + 3072])
                        elif gi == 12:
                            nc.gpsimd.dma_start(
                                y[2 * k + h][:, 9216:NPT],
                                out_sb[:, 9216:NPT])

    nc.finalize()
    _nc_cache["nc"] = nc
    return nc


def _prep_weights(W, b):
    mask = (np.arange(MAX_DIM)[None, None, :] < NODE_DIMS[:, None, None])
    W_eff = np.where(mask, W, 0).astype(np.float32)  # [T, F, D]
    wtb = np.zeros((128, 2 * N_TYPES * 128), dtype=np.float32)
    for k in range(N_TYPES):
        d = int(NODE_DIMS[k])
        for h in range(2):
            c0 = (2 * k + h) * 128
            wtb[0:d, c0:c0 + 128] = W_eff[k, h * 128:(h + 1) * 128, :d].T
            if FOLD[k]:
                wtb[d, c0:c0 + 128] = b[k, h * 128:(h + 1) * 128]
    bvec = np.zeros((128, 4), dtype=np.float32)
    for (k, h), j in BIDX.items():
        bvec[:, j] = b[k, h * 128:(h + 1) * 128]
    return wtb.astype(np.float16), bvec


def _prep_x_shard(x, c):
    """fp16 type-major transposed layout: xd[R_OFF[k]+d, i] = x_k[i, d]
    where x_k[i] = x[c*100000 + 8*i + k] (node i of type k on core c),
    with a ones-row at d = dim_k for the bias-folding types."""
    xc = x[c * NODES_PER_CORE:(c + 1) * NODES_PER_CORE]
    xd = np.zeros((R_TOT, NPT), dtype=np.float16)
    for k in range(N_TYPES):
        d = int(NODE_DIMS[k])
        xk = xc[k::N_TYPES, :d]                       # [12500, d] f32
        xd[R_OFF[k]:R_OFF[k] + d, :NPT_REAL] = xk.astype(np.float16).T
        if FOLD[k]:
            xd[R_OFF[k] + d, :NPT_REAL] = 1.0
    return xd


def run(x, W, b, trace=False):
    nc = _build_nc()
    wtb, bvec = _prep_weights(W, b)
    in_maps = []
    for c in range(N_CORES):
        in_maps.append({
            "xd": _prep_x_shard(x, c),
            "wtb": wtb,
            "bvec": bvec,
        })
    res = run_bass_kernel_spmd(nc, in_maps, list(range(N_CORES)), trace=trace)
    y = np.empty((N_GRAPHS * N_TYPES, FEAT), dtype=np.float32)
    for c in range(N_CORES):
        yd = np.asarray(res.results[c]["y"]).astype(np.float32)
        yd = yd.reshape(N_TYPES, 2, 128, NPT)         # [k, h, p, i]
        yc = yd.transpose(3, 0, 1, 2).reshape(NPT, N_TYPES, FEAT)[:NPT_REAL]
        y[c * NODES_PER_CORE:(c + 1) * NODES_PER_CORE] = yc.reshape(
            NODES_PER_CORE, FEAT)
    return y, res


def kernel(**inputs):
    y, _ = run(inputs["x"], inputs["W"], inputs["b"])
    return y


if __name__ == "__main__":
    rng = np.random.default_rng(0)
    x = rng.standard_normal((N_GRAPHS * N_TYPES, MAX_DIM), dtype=np.float32)
    W = (rng.standard_normal((N_TYPES, FEAT, MAX_DIM), dtype=np.float32) * 0.05)
    b = (rng.standard_normal((N_TYPES, FEAT), dtype=np.float32) * 0.05)
    y, res = run(x, W, b)
    mask = (np.arange(MAX_DIM)[None, None, :] < NODE_DIMS[:, None, None])
    W_eff = np.where(mask, W, 0).astype(np.float32)
    idx = rng.integers(0, N_GRAPHS * N_TYPES, 256)
    exp = np.stack([W_eff[n % 8] @ x[n] + b[n % 8] for n in idx])
    act = y[idx]
    err = np.abs(act - exp).max() / (np.abs(exp).max() + 1e-30)
    print("spot-check rel err:", err)


# revision 9
# speedup vs baseline: 1.2220x; 1.0097x over previous
"""Trainium2 Bass kernel for nn_NodeEncoder (per-type Linear over interleaved node types).

Problem: x [800000, 128] f32, W [8, 256, 128], b [8, 256].
Node n has type k = n % 8; y[n] = (W[k] * mask_k) @ x[n] + b[k], y [800000, 256].

Strategy (8 cores, data-parallel over graphs, weights replicated):
  - Each core gets 100000 consecutive nodes (12500 per type), padded to
    12544 = 28*448 nodes per type.
  - Weight-stationary matmuls: lhsT = W[k] half [kk, 128 feats] (tiny),
    rhs = x streamed [kk, 512 nodes] per chunk, out = PSUM [128 feats,
    512 nodes] fp32 (one full 2 KiB bank).  25 chunks (24x512 + 256)
    per (type, feat-half); 400 matmuls per core total.
  - x ships fp16, type-major: xd[R_OFF[k] + d, i] = x_typek[i, d], each
    type's block fully contiguous -> 8 large input DMAs per core with
    maximal descriptors (25 KiB per partition row).
  - For types with dim < 128 a ones-row is appended (bias rides as an
    extra contraction row of the weight tile).  Types 3,7 (dim 128) get
    their bias during PSUM eviction: ACT activation-bias / DVE
    tensor_scalar_add with a per-partition [128,1] bias vector.
  - Evictions (PSUM fp32 -> SBUF fp16) alternate Scalar:Vector 5:4
    (matching their 1.2 / 0.96 GHz rates).  Output y is written
    feat-major [16 blocks (k,h), 128 feats, 12544 nodes] fp16, each
    (k,h) block one contiguous 3.2 MiB DMA issued from the (otherwise
    idle) GpSimd queue; the host untangles to node-major fp32.
"""

import os
import sys

import numpy as np

for _p in ("/root/.axon_site", "/root/.axon_site/_ro/trn_rl_repo", "/root/.axon_site/_ro/pypackages"):
    if os.path.isdir(_p) and _p not in sys.path:
        sys.path.append(_p)

import concourse.bass as bass
import concourse.mybir as mybir
import concourse.tile as tile
from concourse import bacc
from concourse.bass_utils import run_bass_kernel_spmd

N_TYPES = 8
MAX_DIM = 128
FEAT = 256
N_GRAPHS = 100000
NODE_DIMS = np.array([16, 32, 64, 128, 64, 32, 16, 128], dtype=np.int32)

N_CORES = 8
NODES_PER_CORE = N_GRAPHS * N_TYPES // N_CORES  # 100000
NPT_REAL = NODES_PER_CORE // N_TYPES            # 12500 nodes per type per core
NPT = 12544                                     # padded: 28 * 448 = 24.5 * 512
CHUNKS = [512] * 24 + [256]                     # sum = 12544

_F32 = mybir.dt.float32
_F16 = mybir.dt.float16

# kk = contraction rows per type: dim + 1 (ones-row folds the bias) for
# dim < 128; types 3,7 use all 128 rows and get bias at eviction.
FOLD = [int(d) < MAX_DIM for d in NODE_DIMS]
KK = [int(d) + (1 if f else 0) for d, f in zip(NODE_DIMS, FOLD)]
R_OFF = np.concatenate([[0], np.cumsum(KK)]).astype(int)
R_TOT = int(R_OFF[-1])                          # 486
BIDX = {(3, 0): 0, (3, 1): 1, (7, 0): 2, (7, 1): 3}
TYPE_ORDER = [0, 6, 1, 5, 2, 4, 3, 7]           # small input blocks first

_nc_cache = {}


def _build_nc():
    if "nc" in _nc_cache:
        return _nc_cache["nc"]
    nc = bacc.Bacc("TRN2", target_bir_lowering=False, debug=False)
    xd = nc.dram_tensor("xd", [R_TOT, NPT], _F16, kind="ExternalInput").ap()
    wtb = nc.dram_tensor("wtb", [128, 2 * N_TYPES * 128], _F16, kind="ExternalInput").ap()
    bvec = nc.dram_tensor("bvec", [128, 4], _F32, kind="ExternalInput").ap()
    y = nc.dram_tensor("y", [2 * N_TYPES, 128, NPT], _F16, kind="ExternalOutput").ap()

    ident = mybir.ActivationFunctionType.Identity

    # Per (type, half): 12 PSUM tiles of 2x512-col matmul chunks + a 256-col
    # tail (which borrows a quarter of a pool tile).  One batched eviction
    # per tile; one output DMA per 3 tiles (~786 KiB).
    GROUPS = [(g * 1024, 1024) for g in range(12)] + [(12288, 256)]
    XSPLIT = 6144  # input DMA split point (tile-group boundary)

    with tile.TileContext(nc) as tc:
        with (
            tc.tile_pool(name="const", bufs=1) as const,
            tc.tile_pool(name="xin", bufs=3) as xin_pool,
            tc.tile_pool(name="outsb", bufs=4) as out_pool,
            tc.tile_pool(name="ps", bufs=4, space="PSUM") as ps_pool,
        ):
            wtb_sb = const.tile([128, 2 * N_TYPES * 128], _F16)
            nc.sync.dma_start(wtb_sb[:], wtb[:])
            bv_sb = const.tile([128, 4], _F32)

            ev = 0  # global eviction counter: interleaved 6:5 Scalar:Vector
            for ti, k in enumerate(TYPE_ORDER):
                kk = KK[k]
                xs = xin_pool.tile([128, NPT], _F16, tag="xs", name=f"xs_{k}")
                if ti == 0:
                    # first type: tiny leading chunk so compute starts ASAP
                    nc.sync.dma_start(xs[0:kk, 0:2048],
                                      xd[R_OFF[k]:R_OFF[k] + kk, 0:2048])
                    nc.sync.dma_start(xs[0:kk, 2048:XSPLIT],
                                      xd[R_OFF[k]:R_OFF[k] + kk, 2048:XSPLIT])
                else:
                    nc.sync.dma_start(xs[0:kk, 0:XSPLIT],
                                      xd[R_OFF[k]:R_OFF[k] + kk, 0:XSPLIT])
                nc.sync.dma_start(xs[0:kk, XSPLIT:NPT],
                                  xd[R_OFF[k]:R_OFF[k] + kk, XSPLIT:NPT])
                if ti == 0:
                    nc.sync.dma_start(bv_sb[:], bvec[:])
                for h in range(2):
                    out_sb = out_pool.tile([128, NPT], _F16, tag="os",
                                           name=f"os_{k}_{h}")
                    w_ap = wtb_sb[0:kk, (2 * k + h) * 128:(2 * k + h + 1) * 128]
                    for gi, (goff, gw) in enumerate(GROUPS):
                        ps = ps_pool.tile([128, 1024], _F32, tag="ps",
                                          name=f"ps_{k}_{h}_{gi}")
                        for off in range(0, gw, 512):
                            cw = min(512, gw - off)
                            nc.tensor.matmul(
                                ps[:, off:off + cw], w_ap,
                                xs[0:kk, goff + off:goff + off + cw],
                                start=True, stop=True,
                            )
                        dst = out_sb[:, goff:goff + gw]
                        use_act = (ev % 11) % 2 == 0  # A,D,A,D,... 6:5
                        ev += 1
                        if k in (3, 7):
                            j = BIDX[(k, h)]
                            if use_act:
                                nc.scalar.activation(dst, ps[:, 0:gw], ident,
                                                     bias=bv_sb[:, j:j + 1])
                            else:
                                nc.vector.tensor_scalar_add(dst, ps[:, 0:gw],
                                                            bv_sb[:, j:j + 1])
                        else:
                            if use_act:
                                nc.scalar.copy(dst, ps[:, 0:gw])
                            else:
                                nc.vector.tensor_copy(dst, ps[:, 0:gw])
                        # output DMA per 3 tiles; tail rides with the last.
                        # First (k,h) drains per tile so output starts early.
                        if ti == 0 and h == 0 and gi < 12:
                            nc.gpsimd.dma_start(
                                y[2 * k + h][:, goff:goff + gw],
                                out_sb[:, goff:goff + gw])
                        elif ti == 0 and h == 0:
                            nc.gpsimd.dma_start(
                                y[2 * k + h][:, 12288:NPT],
                                out_sb[:, 12288:NPT])
                        elif gi in (2, 5, 8):
                            doff = goff - 2048
                            nc.gpsimd.dma_start(
                                y[2 * k + h][:, doff:doff + 3072],
                                out_sb[:, doff:doff + 3072])
                        elif gi == 12:
                            nc.gpsimd.dma_start(
                                y[2 * k + h][:, 9216:NPT],
                                out_sb[:, 9216:NPT])

    nc.finalize()
    _nc_cache["nc"] = nc
    return nc


def _prep_weights(W, b):
    mask = (np.arange(MAX_DIM)[None, None, :] < NODE_DIMS[:, None, None])
    W_eff = np.where(mask, W, 0).astype(np.float32)  # [T, F, D]
    wtb = np.zeros((128, 2 * N_TYPES * 128), dtype=np.float32)
    for k in range(N_TYPES):
        d = int(NODE_DIMS[k])
        for h in range(2):
            c0 = (2 * k + h) * 128
            wtb[0:d, c0:c0 + 128] = W_eff[k, h * 128:(h + 1) * 128, :d].T
            if FOLD[k]:
                wtb[d, c0:c0 + 128] = b[k, h * 128:(h + 1) * 128]
    bvec = np.zeros((128, 4), dtype=np.float32)
    for (k, h), j in BIDX.items():
        bvec[:, j] = b[k, h * 128:(h + 1) * 128]
    return wtb.astype(np.float16), bvec


def _prep_x_shard(x, c):
    """fp16 type-major transposed layout: xd[R_OFF[k]+d, i] = x_k[i, d]
    where x_k[i] = x[c*100000 + 8*i + k] (node i of type k on core c),
    with a ones-row at d = dim_k for the bias-folding types."""
    xc = x[c * NODES_PER_CORE:(c + 1) * NODES_PER_CORE]
    xd = np.zeros((R_TOT, NPT), dtype=np.float16)
    for k in range(N_TYPES):
        d = int(NODE_DIMS[k])
        xk = xc[k::N_TYPES, :d]                       # [12500, d] f32
        xd[R_OFF[k]:R_OFF[k] + d, :NPT_REAL] = xk.astype(np.float16).T
        if FOLD[k]:
            xd[R_OFF[k] + d, :NPT_REAL] = 1.0
    return xd


def run(x, W, b, trace=False):
    nc = _build_nc()
    wtb, bvec = _prep_weights(W, b)
    in_maps = []
    for c in range(N_CORES):
        in_maps.append({
            "xd": _prep_x_shard(x, c),
            "wtb": wtb,
            "bvec": bvec,
        })
    res = run_bass_kernel_spmd(nc, in_maps, list(range(N_CORES)), trace=trace)
    y = np.empty((N_GRAPHS * N_TYPES, FEAT), dtype=np.float32)
    for c in range(N_CORES):
        yd = np.asarray(res.results[c]["y"]).astype(np.float32)
        yd = yd.reshape(N_TYPES, 2, 128, NPT)         # [k, h, p, i]
        yc = yd.transpose(3, 0, 1, 2).reshape(NPT, N_TYPES, FEAT)[:NPT_REAL]
        y[c * NODES_PER_CORE:(c + 1) * NODES_PER_CORE] = yc.reshape(
            NODES_PER_CORE, FEAT)
    return y, res


def kernel(**inputs):
    y, _ = run(inputs["x"], inputs["W"], inputs["b"])
    return y


if __name__ == "__main__":
    rng = np.random.default_rng(0)
    x = rng.standard_normal((N_GRAPHS * N_TYPES, MAX_DIM), dtype=np.float32)
    W = (rng.standard_normal((N_TYPES, FEAT, MAX_DIM), dtype=np.float32) * 0.05)
    b = (rng.standard_normal((N_TYPES, FEAT), dtype=np.float32) * 0.05)
    y, res = run(x, W, b)
    mask = (np.arange(MAX_DIM)[None, None, :] < NODE_DIMS[:, None, None])
    W_eff = np.where(mask, W, 0).astype(np.float32)
    idx = rng.integers(0, N_GRAPHS * N_TYPES, 256)
    exp = np.stack([W_eff[n % 8] @ x[n] + b[n % 8] for n in idx])
    act = y[idx]
    err = np.abs(act - exp).max() / (np.abs(exp).max() + 1e-30)
    print("spot-check rel err:", err)
